# revision 15
# baseline (speedup 1.0000x reference)
"""MoE routing kernel for TRN2 (8 NeuronCores), Bass/Tile.

Data-parallel over samples with a routing-specialized fully-static PE
schedule. Host computes gating (bit-exact jnp ops), then deals samples to
cores by global distinct-expert-count rank so region q of every core has
the same static size R[q] (NSLOT = sum(R) == ceil(total_distinct/8), i.e.
optimal). Per (sample, expert) "slot" the device computes:

    h1 = W1[e] @ x[s] + b1          (mm1, PSUM -> fp16 SBUF)
    h2 = relu(W2[e] @ h1 + bAp)     (mm2, PSUM -> fp16 SBUF)
    z  = (W3[e]*inv) @ h2           (mm3, PSUM -> fp16 SBUF -> HBM)

One slot is shared by every (gate, t) instance that routes sample s to
expert e (z-dedup: ~5.4 of 8 instances distinct -> 44 slots/core instead
of 64). The per-(gate,sample) combine y = tw0*z0 + tw1*z1 + tw.b3 is 0.4%
of the FLOPs and pure routing arithmetic; it runs on the host together
with the gating, so the device program is 100% static: the PE stream is
only LDWEIGHTS + MATMUL, drains are balanced across Scalar/Vector, DMAs
are chunk-batched on the Sync/GpSimd queues, and a short warmup matmul
burst keeps PE busy from t~0 so the HAM clock gate is released before
real matmuls arrive. mm3 of slot d is scheduled 2 slots late so its h2
dependency is always long-satisfied.

The Tile program depends only on the region-size vector R (lru-cached;
inputs are deterministic per problem, so it compiles once)."""
import functools

import numpy as np

E, TOP, C, HD, B, H, W_, NG = 8, 2, 128, 256, 64, 32, 32, 4
P = H * W_            # 1024
NCORES = 8
SPC = B // NCORES     # samples (== regions) per core: 8
EPS = 1e-5
NH = 512              # matmul free-dim chunk (one PSUM bank)
WSC = 768             # ws panel cols: W1T(256) | W2T_k0(256) | W2T_k1(256)
N_WARM = 5            # warmup matmuls (cover initial DMA wait, warm HAM)


def _chunks(total, sizes=(1, 1, 2, 4, 4, 8, 8, 8, 8, 8, 8)):
    """Split `total` slots into DMA chunks, small chunks first."""
    out, i = [], 0
    while total > 0:
        s = min(sizes[min(i, len(sizes) - 1)], total)
        out.append(s)
        total -= s
        i += 1
    return out


XQ_CHUNKS = (1, 1, 2, 4)  # region chunks for x loads


@functools.lru_cache(maxsize=2)
def _build_program(Rkey):
    from concourse import bacc, mybir
    import concourse.tile as tile

    R = list(Rkey)
    NSLOT = sum(R)
    WCH = _chunks(NSLOT)
    f32 = mybir.dt.float32
    f16 = mybir.dt.float16
    MCOLS = 4 * NSLOT
    nc = bacc.Bacc("TRN2", target_bir_lowering=False, debug=False)

    slot_region = []
    for q, r in enumerate(R):
        slot_region += [q] * r
    xq_chunk_of = []  # region -> (chunk idx, local idx)
    for ci_, n in enumerate(XQ_CHUNKS):
        for li in range(n):
            xq_chunk_of.append((ci_, li))
    ws_chunk_of = []
    for ci_, n in enumerate(WCH):
        for li in range(n):
            ws_chunk_of.append((ci_, li))

    xq_d = [nc.dram_tensor(f"xq{i}", [C, n * P], f16, kind="ExternalInput")
            for i, n in enumerate(XQ_CHUNKS)]
    ws_d = [nc.dram_tensor(f"ws{i}", [C, n * WSC], f16, kind="ExternalInput")
            for i, n in enumerate(WCH)]
    w3_d = [nc.dram_tensor(f"w3{i}", [C, n * 256], f16, kind="ExternalInput")
            for i, n in enumerate(WCH)]
    meta_d = nc.dram_tensor("meta", [C, MCOLS], f32, kind="ExternalInput")
    out_d = nc.dram_tensor("out", [NSLOT, C, P], f16, kind="ExternalOutput")

    AOP = mybir.AluOpType

    with tile.TileContext(nc) as tc:
        with tc.tile_pool(name="per", bufs=1) as per, \
             tc.tile_pool(name="h1p", bufs=2) as h1pool, \
             tc.tile_pool(name="h2p", bufs=3) as h2pool, \
             tc.tile_pool(name="zbp", bufs=3) as zpool, \
             tc.tile_pool(name="ps", bufs=4, space="PSUM") as pspool:

            # ---- persistent tiles ----
            warm = per.tile([C, NH], f16, tag="warm", name="warm")
            meta = per.tile([C, MCOLS], f32, tag="meta", name="meta")
            xq_t = [per.tile([C, n * P], f16, tag=f"xq{i}", name=f"xqt{i}")
                    for i, n in enumerate(XQ_CHUNKS)]
            ws_t = [per.tile([C, n * WSC], f16, tag=f"ws{i}", name=f"wst{i}")
                    for i, n in enumerate(WCH)]
            w3_t = [per.tile([C, n * 256], f16, tag=f"w3{i}", name=f"w3t{i}")
                    for i, n in enumerate(WCH)]

            # ---- warmup: PE busy from ~t0 while DMAs land ----
            nc.vector.memset(warm[:], 0.0)
            ps_w = pspool.tile([C, P], f32, tag="ps", name="ps_warm")
            for i in range(N_WARM):
                nc.tensor.matmul(ps_w[:, (i % 2) * NH:(i % 2 + 1) * NH],
                                 warm[:, 0:128], warm[:], start=True,
                                 stop=True)

            # ---- input DMAs (sync queue: ws/x/meta; gpsimd: w3),
            # criticality order: slot 0's panel gates the first real MM.
            order = [("ws", 0), ("xq", 0), ("w3", 0), ("meta", 0)]
            for i in range(1, max(len(XQ_CHUNKS), len(WCH))):
                if i < len(WCH):
                    order.append(("ws", i))
                if i < len(XQ_CHUNKS):
                    order.append(("xq", i))
                if i < len(WCH):
                    order.append(("w3", i))
            for kind, i in order:
                if kind == "meta":
                    nc.sync.dma_start(out=meta[:], in_=meta_d[:])
                    continue
                if kind == "xq":
                    nc.sync.dma_start(out=xq_t[i][:], in_=xq_d[i][:])
                elif kind == "ws":
                    nc.sync.dma_start(out=ws_t[i][:], in_=ws_d[i][:])
                else:
                    nc.gpsimd.dma_start(out=w3_t[i][:], in_=w3_d[i][:])

            # drain-engine balancer: pick engine with least queued time
            eng_load = {"act": 0.0, "dve": 0.0}

            def drain(out_ap, in_ap, bias_ap, relu):
                a, v = eng_load["act"], eng_load["dve"]
                if a + 1.11 <= v + 1.27:
                    nc.scalar.activation(
                        out=out_ap, in_=in_ap,
                        func=(mybir.ActivationFunctionType.Relu if relu else
                              mybir.ActivationFunctionType.Identity),
                        bias=bias_ap, scale=1.0)
                    eng_load["act"] = a + 1.11
                else:
                    if relu:
                        nc.vector.tensor_scalar(
                            out=out_ap, in0=in_ap, scalar1=bias_ap,
                            scalar2=0.0, op0=AOP.add, op1=AOP.max)
                    else:
                        nc.vector.tensor_scalar_add(
                            out=out_ap, in0=in_ap, scalar1=bias_ap)
                    eng_load["dve"] = v + 1.27

            h1sb = {}   # d -> h1 sbuf tile
            h2sb = {}   # d -> h2 sbuf tile

            def emit_mm1(d):
                q = slot_region[d]
                xci, xli = xq_chunk_of[q]
                wci, wli = ws_chunk_of[d]
                xt = xq_t[xci]
                wt = ws_t[wci]
                wb = wli * WSC
                xb = xli * P
                psA = [pspool.tile([C, P], f32, tag="ps", name=f"psA{m}_{d}")
                       for m in range(2)]
                h1t = h1pool.tile([C, 2 * P], f16, tag="h1", name=f"h1_{d}")
                for m in range(2):
                    lhs = wt[:, wb + m * 128:wb + (m + 1) * 128]
                    for n in range(2):
                        nc.tensor.matmul(
                            psA[m][:, n * NH:(n + 1) * NH], lhs,
                            xt[:, xb + n * NH:xb + (n + 1) * NH],
                            start=True, stop=True)
                    drain(h1t[:, m * P:(m + 1) * P], psA[m][:],
                          meta[:, 4 * d + m:4 * d + m + 1], False)
                h1sb[d] = h1t

            def emit_mm2(d):
                wci, wli = ws_chunk_of[d]
                wt = ws_t[wci]
                wb = wli * WSC
                h1t = h1sb.pop(d)
                psB = [pspool.tile([C, P], f32, tag="ps", name=f"psB{m}_{d}")
                       for m in range(2)]
                h2t = h2pool.tile([C, 2 * P], f16, tag="h2", name=f"h2_{d}")
                for m in range(2):       # m-outer: frees psB m0 earlier
                    for k in range(2):
                        lhs = wt[:, wb + 256 + k * 256 + m * 128:
                                 wb + 256 + k * 256 + (m + 1) * 128]
                        for n in range(2):
                            nc.tensor.matmul(
                                psB[m][:, n * NH:(n + 1) * NH], lhs,
                                h1t[:, k * P + n * NH:k * P + (n + 1) * NH],
                                start=(k == 0), stop=(k == 1))
                    drain(h2t[:, m * P:(m + 1) * P], psB[m][:],
                          meta[:, 4 * d + 2 + m:4 * d + 3 + m], True)
                h2sb[d] = h2t

            def emit_mm3z(d, tail=False):
                wci, wli = ws_chunk_of[d]
                w3b = wli * 256
                w3t = w3_t[wci]
                h2t = h2sb.pop(d)
                psZ = pspool.tile([C, P], f32, tag="ps", name=f"psZ_{d}")
                for k in range(2):
                    lhs = w3t[:, w3b + k * 128:w3b + (k + 1) * 128]
                    for n in range(2):
                        nc.tensor.matmul(
                            psZ[:, n * NH:(n + 1) * NH], lhs,
                            h2t[:, k * P + n * NH:k * P + (n + 1) * NH],
                            start=(k == 0), stop=(k == 1))
                zt = zpool.tile([C, P], f16, tag="zb", name=f"z_{d}")
                if tail:  # final slots: halve latency by using both engines
                    nc.scalar.activation(
                        out=zt[:, 0:NH], in_=psZ[:, 0:NH],
                        func=mybir.ActivationFunctionType.Identity,
                        bias=0.0, scale=1.0)
                    nc.vector.tensor_scalar_add(
                        out=zt[:, NH:P], in0=psZ[:, NH:P], scalar1=0.0)
                    nc.gpsimd.dma_start(out=out_d[d][:, 0:NH],
                                        in_=zt[:, 0:NH])
                    nc.sync.dma_start(out=out_d[d][:, NH:P],
                                      in_=zt[:, NH:P])
                else:
                    drain(zt[:], psZ[:], 0.0, False)
                    nc.gpsimd.dma_start(out=out_d[d], in_=zt[:])

            # ---- main loop, staggered so every drain has >1.5us slack
            # before its consumer and every engine FIFO is enqueued in
            # readiness order: iter i runs mm1(i), mm3(i-3), mm2(i-1).
            for i in range(NSLOT + 3):
                if i < NSLOT:
                    emit_mm1(i)
                if i >= 3:
                    emit_mm3z(i - 3, tail=(i - 3 >= NSLOT - 2))
                if 1 <= i <= NSLOT:
                    emit_mm2(i - 1)

    nc.compile()
    return nc


def _gating(x, gates):
    """Host gating, eager jnp op-for-op as the reference (bit-exact)."""
    import jax
    import jax.numpy as jnp

    xj = jnp.asarray(x)
    gj = jnp.asarray(gates)
    x0 = xj.mean(axis=(2, 3))                      # [B, C]
    tis, tws = [], []
    for i in range(NG):
        probs = jax.nn.softmax(x0 @ gj[i], axis=1)  # [B, E]
        top_p, top_i = jax.lax.top_k(probs, TOP)    # [B, TOP]
        tw = jax.nn.softmax(top_p, axis=1)          # [B, TOP]
        tis.append(np.asarray(top_i))
        tws.append(np.asarray(tw).astype(np.float32))
    return np.stack(tis), np.stack(tws)


def _np_fallback(inputs, top_i, tw):
    """Pure-numpy reference path (degenerate bn params only)."""
    x = np.asarray(inputs["x"], np.float32).reshape(B, C, P)
    W1 = np.asarray(inputs["W1"], np.float32)
    b1 = np.asarray(inputs["b1"], np.float32)
    W2 = np.asarray(inputs["W2"], np.float32)
    b2 = np.asarray(inputs["b2"], np.float32)
    gmm = np.asarray(inputs["bn_gamma"], np.float32)
    bet = np.asarray(inputs["bn_beta"], np.float32)
    mea = np.asarray(inputs["bn_mean"], np.float32)
    var = np.asarray(inputs["bn_var"], np.float32)
    W3 = np.asarray(inputs["W3"], np.float32)
    b3 = np.asarray(inputs["b3"], np.float32)
    inv = gmm / np.sqrt(var + EPS)
    outs = []
    for g in range(NG):
        og = np.zeros((B, C, P), np.float32)
        for b in range(B):
            for t in range(TOP):
                e = int(top_i[g, b, t])
                w = tw[g, b, t]
                h = W1[e] @ x[b] + b1[e][:, None]
                h = W2[e] @ h + b2[e][:, None]
                h = np.maximum(
                    (h - mea[e][:, None]) * inv[e][:, None]
                    + bet[e][:, None], 0.0)
                og[b] += w * (W3[e] @ h + b3[e][:, None])
        outs.append(og.reshape(B, C, H, W_))
    return tuple(outs)


def build_in_maps(inputs):
    """Gating, schedule, packed fp16 panels, per-core input maps.

    Returns ((in_maps, schedule), None) or (None, outputs)."""
    x = np.asarray(inputs["x"], dtype=np.float32)
    gates = np.asarray(inputs["gates"], dtype=np.float32)
    W1 = np.asarray(inputs["W1"], dtype=np.float32)
    b1 = np.asarray(inputs["b1"], dtype=np.float32)
    W2 = np.asarray(inputs["W2"], dtype=np.float32)
    b2 = np.asarray(inputs["b2"], dtype=np.float32)
    bn_gamma = np.asarray(inputs["bn_gamma"], dtype=np.float32)
    bn_beta = np.asarray(inputs["bn_beta"], dtype=np.float32)
    bn_mean = np.asarray(inputs["bn_mean"], dtype=np.float32)
    bn_var = np.asarray(inputs["bn_var"], dtype=np.float32)
    W3 = np.asarray(inputs["W3"], dtype=np.float32)
    b3 = np.asarray(inputs["b3"], dtype=np.float32)

    top_i, tw = _gating(x, gates)  # [NG,B,TOP]

    inv = bn_gamma / np.sqrt(bn_var + np.float32(EPS))   # [E, HD]
    if not np.all(inv > 0):
        return None, _np_fallback(inputs, top_i, tw)
    biasA = (b2 - bn_mean) * inv + bn_beta               # [E, HD]
    bAp = (biasA / inv).astype(np.float32)               # [E, HD]

    # per-sample distinct expert sets (first-appearance order)
    esets = []
    for s in range(B):
        seen = []
        for g in range(NG):
            for t in range(TOP):
                e = int(top_i[g, s, t])
                if e not in seen:
                    seen.append(e)
        esets.append(seen)
    dcount = np.array([len(s) for s in esets])

    # deal samples to cores by global rank: region q of core c gets
    # ranks[SPC*q + c]; R[q] = max demand in that rank row.
    ranks = np.argsort(-dcount, kind="stable")
    Rl = [int(max(dcount[ranks[SPC * q + c]] for c in range(NCORES)))
          for q in range(SPC)]
    Rkey = tuple(Rl)
    NSLOT = sum(Rl)
    WCH = _chunks(NSLOT)
    MCOLS = 4 * NSLOT

    # packed fp16 expert panels
    wpanel = np.empty((E, C, WSC), dtype=np.float16)
    w3pan = np.empty((E, C, 256), dtype=np.float16)
    for e in range(E):
        wpanel[e, :, 0:256] = W1[e].T
        w2t = W2[e].T
        wpanel[e, :, 256:512] = w2t[0:128, :]
        wpanel[e, :, 512:768] = w2t[128:256, :]
        w3inv = (W3[e] * inv[e][None, :]).T        # [HD, C]
        w3pan[e, :, 0:128] = w3inv[0:128, :]
        w3pan[e, :, 128:256] = w3inv[128:256, :]

    xr = x.reshape(B, C, P)
    in_maps = []
    orders = []      # core -> region -> sample
    slot_maps = []   # core -> {(sample, expert): slot}
    for c in range(NCORES):
        order = [int(ranks[SPC * q + c]) for q in range(SPC)]
        orders.append(order)
        slots = []
        slot_of = {}
        for q in range(SPC):
            s = order[q]
            es = esets[s] + [esets[s][0]] * (Rl[q] - len(esets[s]))
            for r, e in enumerate(es):
                if r < len(esets[s]):
                    slot_of[(s, e)] = len(slots)
                slots.append(e)
        slot_maps.append(slot_of)

        ws = np.empty((C, NSLOT * WSC), dtype=np.float16)
        w3 = np.empty((C, NSLOT * 256), dtype=np.float16)
        meta = np.zeros((C, MCOLS), dtype=np.float32)
        for d, e in enumerate(slots):
            ws[:, d * WSC:(d + 1) * WSC] = wpanel[e]
            w3[:, d * 256:(d + 1) * 256] = w3pan[e]
            meta[:, 4 * d + 0] = b1[e, 0:128]
            meta[:, 4 * d + 1] = b1[e, 128:256]
            meta[:, 4 * d + 2] = bAp[e, 0:128]
            meta[:, 4 * d + 3] = bAp[e, 128:256]

        im = {"meta": meta}
        base = 0
        for i, n in enumerate(XQ_CHUNKS):
            xc = np.empty((C, n * P), dtype=np.float16)
            for li in range(n):
                xc[:, li * P:(li + 1) * P] = xr[order[base + li]]
            im[f"xq{i}"] = xc
            base += n
        base = 0
        for i, n in enumerate(WCH):
            im[f"ws{i}"] = np.ascontiguousarray(
                ws[:, base * WSC:(base + n) * WSC])
            im[f"w3{i}"] = np.ascontiguousarray(
                w3[:, base * 256:(base + n) * 256])
            base += n
        in_maps.append(im)
    sched = (orders, slot_maps, Rkey, top_i, tw, b3)
    return (in_maps, sched), None


def combine_outputs(results, sched):
    """Host combine: y[g,s] = tw0*z[s,e0] + tw1*z[s,e1] + tw.b3."""
    orders, slot_maps, _Rkey, top_i, tw, b3 = sched
    core_of = {}
    for c in range(NCORES):
        for s in orders[c]:
            core_of[s] = c
    zs = [np.asarray(r["out"], dtype=np.float32) for r in results]
    outs = []
    for g in range(NG):
        og = np.empty((B, C, P), dtype=np.float32)
        for s in range(B):
            c = core_of[s]
            y = None
            for t in range(TOP):
                e = int(top_i[g, s, t])
                w = float(tw[g, s, t])
                zt = zs[c][slot_maps[c][(s, e)]]
                y = w * zt if y is None else y + w * zt
                if np.any(b3[e]):
                    y = y + w * b3[e][:, None]
            og[s] = y
        outs.append(og.reshape(B, C, H, W_))
    return tuple(outs)


def kernel(x, gates, W1, b1, W2, b2, bn_gamma, bn_beta, bn_mean, bn_var,
           W3, b3):
    from concourse.bass_utils import run_bass_kernel_spmd

    built, fb = build_in_maps({
        "x": x, "gates": gates, "W1": W1, "b1": b1, "W2": W2, "b2": b2,
        "bn_gamma": bn_gamma, "bn_beta": bn_beta, "bn_mean": bn_mean,
        "bn_var": bn_var, "W3": W3, "b3": b3,
    })
    if fb is not None:
        return fb
    in_maps, sched = built
    nc = _build_program(sched[2])
    res = run_bass_kernel_spmd(nc, in_maps, list(range(NCORES)))
    return combine_outputs(res.results, sched)


# revision 16
# speedup vs baseline: 1.0100x; 1.0100x over previous
"""MoE routing kernel for TRN2 (8 NeuronCores), Bass/Tile.

Data-parallel over samples with a routing-specialized fully-static PE
schedule. Host computes gating (bit-exact jnp ops), then deals samples to
cores by global distinct-expert-count rank so region q of every core has
the same static size R[q] (NSLOT = sum(R) == ceil(total_distinct/8), i.e.
optimal). Per (sample, expert) "slot" the device computes:

    h1 = W1[e] @ x[s] + b1          (mm1, PSUM -> fp16 SBUF)
    h2 = relu(W2[e] @ h1 + bAp)     (mm2, PSUM -> fp16 SBUF)
    z  = (W3[e]*inv) @ h2           (mm3, PSUM -> fp16 SBUF -> HBM)

One slot is shared by every (gate, t) instance that routes sample s to
expert e (z-dedup: ~5.4 of 8 instances distinct -> 44 slots/core instead
of 64). The per-(gate,sample) combine y = tw0*z0 + tw1*z1 + tw.b3 is 0.4%
of the FLOPs and pure routing arithmetic; it runs on the host together
with the gating, so the device program is 100% static: the PE stream is
only LDWEIGHTS + MATMUL, drains are balanced across Scalar/Vector, DMAs
are chunk-batched on the Sync/GpSimd queues, and a short warmup matmul
burst keeps PE busy from t~0 so the HAM clock gate is released before
real matmuls arrive. mm3 of slot d is scheduled 2 slots late so its h2
dependency is always long-satisfied.

The Tile program depends only on the region-size vector R (lru-cached;
inputs are deterministic per problem, so it compiles once)."""
import functools

import numpy as np

E, TOP, C, HD, B, H, W_, NG = 8, 2, 128, 256, 64, 32, 32, 4
P = H * W_            # 1024
NCORES = 8
SPC = B // NCORES     # samples (== regions) per core: 8
EPS = 1e-5
NH = 512              # matmul free-dim chunk (one PSUM bank)
WSC = 768             # ws panel cols: W1T(256) | W2T_k0(256) | W2T_k1(256)
N_WARM = 11           # warmup matmuls (cover initial DMA wait, warm HAM)


def _chunks(total, sizes=(1, 1, 2, 4, 4, 8, 8, 8, 8, 8, 8)):
    """Split `total` slots into DMA chunks, small chunks first."""
    out, i = [], 0
    while total > 0:
        s = min(sizes[min(i, len(sizes) - 1)], total)
        out.append(s)
        total -= s
        i += 1
    return out


XQ_CHUNKS = (1, 1, 2, 4)  # region chunks for x loads


@functools.lru_cache(maxsize=2)
def _build_program(Rkey):
    from concourse import bacc, mybir
    import concourse.tile as tile

    R = list(Rkey)
    NSLOT = sum(R)
    WCH = _chunks(NSLOT)
    f32 = mybir.dt.float32
    f16 = mybir.dt.float16
    MCOLS = 4 * NSLOT
    nc = bacc.Bacc("TRN2", target_bir_lowering=False, debug=False)

    slot_region = []
    for q, r in enumerate(R):
        slot_region += [q] * r
    xq_chunk_of = []  # region -> (chunk idx, local idx)
    for ci_, n in enumerate(XQ_CHUNKS):
        for li in range(n):
            xq_chunk_of.append((ci_, li))
    ws_chunk_of = []
    for ci_, n in enumerate(WCH):
        for li in range(n):
            ws_chunk_of.append((ci_, li))

    xq_d = [nc.dram_tensor(f"xq{i}", [C, n * P], f16, kind="ExternalInput")
            for i, n in enumerate(XQ_CHUNKS)]
    ws_d = [nc.dram_tensor(f"ws{i}", [C, n * WSC], f16, kind="ExternalInput")
            for i, n in enumerate(WCH)]
    w3_d = [nc.dram_tensor(f"w3{i}", [C, n * 256], f16, kind="ExternalInput")
            for i, n in enumerate(WCH)]
    meta_d = nc.dram_tensor("meta", [C, MCOLS], f32, kind="ExternalInput")
    out_d = nc.dram_tensor("out", [NSLOT, C, P], f16, kind="ExternalOutput")

    AOP = mybir.AluOpType

    with tile.TileContext(nc) as tc:
        with tc.tile_pool(name="per", bufs=1) as per, \
             tc.tile_pool(name="h1p", bufs=2) as h1pool, \
             tc.tile_pool(name="h2p", bufs=3) as h2pool, \
             tc.tile_pool(name="zbp", bufs=3) as zpool, \
             tc.tile_pool(name="ps", bufs=4, space="PSUM") as pspool:

            # ---- persistent tiles ----
            warm = per.tile([C, NH], f16, tag="warm", name="warm")
            meta = per.tile([C, MCOLS], f32, tag="meta", name="meta")
            xq_t = [per.tile([C, n * P], f16, tag=f"xq{i}", name=f"xqt{i}")
                    for i, n in enumerate(XQ_CHUNKS)]
            ws_t = [per.tile([C, n * WSC], f16, tag=f"ws{i}", name=f"wst{i}")
                    for i, n in enumerate(WCH)]
            w3_t = [per.tile([C, n * 256], f16, tag=f"w3{i}", name=f"w3t{i}")
                    for i, n in enumerate(WCH)]

            # ---- warmup: PE busy from ~t0 while DMAs land ----
            nc.vector.memset(warm[:], 0.0)
            ps_w = pspool.tile([C, P], f32, tag="ps", name="ps_warm")
            for i in range(N_WARM):
                nc.tensor.matmul(ps_w[:, (i % 2) * NH:(i % 2 + 1) * NH],
                                 warm[:, 0:128], warm[:], start=True,
                                 stop=True)

            # ---- input DMAs (sync queue: ws/x/meta; gpsimd: w3),
            # criticality order: slot 0's panel gates the first real MM.
            order = [("ws", 0), ("xq", 0), ("w3", 0), ("meta", 0)]
            for i in range(1, max(len(XQ_CHUNKS), len(WCH))):
                if i < len(WCH):
                    order.append(("ws", i))
                if i < len(XQ_CHUNKS):
                    order.append(("xq", i))
                if i < len(WCH):
                    order.append(("w3", i))
            for kind, i in order:
                if kind == "meta":
                    nc.sync.dma_start(out=meta[:], in_=meta_d[:])
                    continue
                if kind == "xq":
                    nc.sync.dma_start(out=xq_t[i][:], in_=xq_d[i][:])
                elif kind == "ws":
                    nc.sync.dma_start(out=ws_t[i][:], in_=ws_d[i][:])
                else:
                    nc.gpsimd.dma_start(out=w3_t[i][:], in_=w3_d[i][:])

            # drain-engine balancer: pick engine with least queued time
            eng_load = {"act": 0.0, "dve": 0.0}

            def drain(out_ap, in_ap, bias_ap, relu):
                a, v = eng_load["act"], eng_load["dve"]
                if a + 1.11 <= v + 1.27:
                    nc.scalar.activation(
                        out=out_ap, in_=in_ap,
                        func=(mybir.ActivationFunctionType.Relu if relu else
                              mybir.ActivationFunctionType.Identity),
                        bias=bias_ap, scale=1.0)
                    eng_load["act"] = a + 1.11
                else:
                    if relu:
                        nc.vector.tensor_scalar(
                            out=out_ap, in0=in_ap, scalar1=bias_ap,
                            scalar2=0.0, op0=AOP.add, op1=AOP.max)
                    else:
                        nc.vector.tensor_scalar_add(
                            out=out_ap, in0=in_ap, scalar1=bias_ap)
                    eng_load["dve"] = v + 1.27

            h1sb = {}   # d -> h1 sbuf tile
            h2sb = {}   # d -> h2 sbuf tile

            def emit_mm1(d):
                q = slot_region[d]
                xci, xli = xq_chunk_of[q]
                wci, wli = ws_chunk_of[d]
                xt = xq_t[xci]
                wt = ws_t[wci]
                wb = wli * WSC
                xb = xli * P
                psA = [pspool.tile([C, P], f32, tag="ps", name=f"psA{m}_{d}")
                       for m in range(2)]
                h1t = h1pool.tile([C, 2 * P], f16, tag="h1", name=f"h1_{d}")
                for m in range(2):
                    lhs = wt[:, wb + m * 128:wb + (m + 1) * 128]
                    for n in range(2):
                        nc.tensor.matmul(
                            psA[m][:, n * NH:(n + 1) * NH], lhs,
                            xt[:, xb + n * NH:xb + (n + 1) * NH],
                            start=True, stop=True)
                    drain(h1t[:, m * P:(m + 1) * P], psA[m][:],
                          meta[:, 4 * d + m:4 * d + m + 1], False)
                h1sb[d] = h1t

            def emit_mm2(d):
                wci, wli = ws_chunk_of[d]
                wt = ws_t[wci]
                wb = wli * WSC
                h1t = h1sb.pop(d)
                psB = [pspool.tile([C, P], f32, tag="ps", name=f"psB{m}_{d}")
                       for m in range(2)]
                h2t = h2pool.tile([C, 2 * P], f16, tag="h2", name=f"h2_{d}")
                for m in range(2):       # m-outer: frees psB m0 earlier
                    for k in range(2):
                        lhs = wt[:, wb + 256 + k * 256 + m * 128:
                                 wb + 256 + k * 256 + (m + 1) * 128]
                        for n in range(2):
                            nc.tensor.matmul(
                                psB[m][:, n * NH:(n + 1) * NH], lhs,
                                h1t[:, k * P + n * NH:k * P + (n + 1) * NH],
                                start=(k == 0), stop=(k == 1))
                    drain(h2t[:, m * P:(m + 1) * P], psB[m][:],
                          meta[:, 4 * d + 2 + m:4 * d + 3 + m], True)
                h2sb[d] = h2t

            def emit_mm3z(d, tail=False):
                wci, wli = ws_chunk_of[d]
                w3b = wli * 256
                w3t = w3_t[wci]
                h2t = h2sb.pop(d)
                psZ = pspool.tile([C, P], f32, tag="ps", name=f"psZ_{d}")
                for k in range(2):
                    lhs = w3t[:, w3b + k * 128:w3b + (k + 1) * 128]
                    for n in range(2):
                        nc.tensor.matmul(
                            psZ[:, n * NH:(n + 1) * NH], lhs,
                            h2t[:, k * P + n * NH:k * P + (n + 1) * NH],
                            start=(k == 0), stop=(k == 1))
                zt = zpool.tile([C, P], f16, tag="zb", name=f"z_{d}")
                if tail:  # final slots: halve latency by using both engines
                    nc.scalar.activation(
                        out=zt[:, 0:NH], in_=psZ[:, 0:NH],
                        func=mybir.ActivationFunctionType.Identity,
                        bias=0.0, scale=1.0)
                    nc.vector.tensor_scalar_add(
                        out=zt[:, NH:P], in0=psZ[:, NH:P], scalar1=0.0)
                    nc.gpsimd.dma_start(out=out_d[d][:, 0:NH],
                                        in_=zt[:, 0:NH])
                    nc.sync.dma_start(out=out_d[d][:, NH:P],
                                      in_=zt[:, NH:P])
                else:
                    drain(zt[:], psZ[:], 0.0, False)
                    nc.gpsimd.dma_start(out=out_d[d], in_=zt[:])

            # ---- main loop, staggered so every drain has >1.5us slack
            # before its consumer and every engine FIFO is enqueued in
            # readiness order: iter i runs mm1(i), mm3(i-3), mm2(i-1).
            for i in range(NSLOT + 3):
                if i < NSLOT:
                    emit_mm1(i)
                if i >= 3:
                    emit_mm3z(i - 3, tail=(i - 3 >= NSLOT - 2))
                if 1 <= i <= NSLOT:
                    emit_mm2(i - 1)

    nc.compile()
    return nc


def _gating(x, gates):
    """Host gating, eager jnp op-for-op as the reference (bit-exact)."""
    import jax
    import jax.numpy as jnp

    xj = jnp.asarray(x)
    gj = jnp.asarray(gates)
    x0 = xj.mean(axis=(2, 3))                      # [B, C]
    tis, tws = [], []
    for i in range(NG):
        probs = jax.nn.softmax(x0 @ gj[i], axis=1)  # [B, E]
        top_p, top_i = jax.lax.top_k(probs, TOP)    # [B, TOP]
        tw = jax.nn.softmax(top_p, axis=1)          # [B, TOP]
        tis.append(np.asarray(top_i))
        tws.append(np.asarray(tw).astype(np.float32))
    return np.stack(tis), np.stack(tws)


def _np_fallback(inputs, top_i, tw):
    """Pure-numpy reference path (degenerate bn params only)."""
    x = np.asarray(inputs["x"], np.float32).reshape(B, C, P)
    W1 = np.asarray(inputs["W1"], np.float32)
    b1 = np.asarray(inputs["b1"], np.float32)
    W2 = np.asarray(inputs["W2"], np.float32)
    b2 = np.asarray(inputs["b2"], np.float32)
    gmm = np.asarray(inputs["bn_gamma"], np.float32)
    bet = np.asarray(inputs["bn_beta"], np.float32)
    mea = np.asarray(inputs["bn_mean"], np.float32)
    var = np.asarray(inputs["bn_var"], np.float32)
    W3 = np.asarray(inputs["W3"], np.float32)
    b3 = np.asarray(inputs["b3"], np.float32)
    inv = gmm / np.sqrt(var + EPS)
    outs = []
    for g in range(NG):
        og = np.zeros((B, C, P), np.float32)
        for b in range(B):
            for t in range(TOP):
                e = int(top_i[g, b, t])
                w = tw[g, b, t]
                h = W1[e] @ x[b] + b1[e][:, None]
                h = W2[e] @ h + b2[e][:, None]
                h = np.maximum(
                    (h - mea[e][:, None]) * inv[e][:, None]
                    + bet[e][:, None], 0.0)
                og[b] += w * (W3[e] @ h + b3[e][:, None])
        outs.append(og.reshape(B, C, H, W_))
    return tuple(outs)


def build_in_maps(inputs):
    """Gating, schedule, packed fp16 panels, per-core input maps.

    Returns ((in_maps, schedule), None) or (None, outputs)."""
    x = np.asarray(inputs["x"], dtype=np.float32)
    gates = np.asarray(inputs["gates"], dtype=np.float32)
    W1 = np.asarray(inputs["W1"], dtype=np.float32)
    b1 = np.asarray(inputs["b1"], dtype=np.float32)
    W2 = np.asarray(inputs["W2"], dtype=np.float32)
    b2 = np.asarray(inputs["b2"], dtype=np.float32)
    bn_gamma = np.asarray(inputs["bn_gamma"], dtype=np.float32)
    bn_beta = np.asarray(inputs["bn_beta"], dtype=np.float32)
    bn_mean = np.asarray(inputs["bn_mean"], dtype=np.float32)
    bn_var = np.asarray(inputs["bn_var"], dtype=np.float32)
    W3 = np.asarray(inputs["W3"], dtype=np.float32)
    b3 = np.asarray(inputs["b3"], dtype=np.float32)

    top_i, tw = _gating(x, gates)  # [NG,B,TOP]

    inv = bn_gamma / np.sqrt(bn_var + np.float32(EPS))   # [E, HD]
    if not np.all(inv > 0):
        return None, _np_fallback(inputs, top_i, tw)
    biasA = (b2 - bn_mean) * inv + bn_beta               # [E, HD]
    bAp = (biasA / inv).astype(np.float32)               # [E, HD]

    # per-sample distinct expert sets (first-appearance order)
    esets = []
    for s in range(B):
        seen = []
        for g in range(NG):
            for t in range(TOP):
                e = int(top_i[g, s, t])
                if e not in seen:
                    seen.append(e)
        esets.append(seen)
    dcount = np.array([len(s) for s in esets])

    # deal samples to cores by global rank: region q of core c gets
    # ranks[SPC*q + c]; R[q] = max demand in that rank row.
    ranks = np.argsort(-dcount, kind="stable")
    Rl = [int(max(dcount[ranks[SPC * q + c]] for c in range(NCORES)))
          for q in range(SPC)]
    Rkey = tuple(Rl)
    NSLOT = sum(Rl)
    WCH = _chunks(NSLOT)
    MCOLS = 4 * NSLOT

    # packed fp16 expert panels
    wpanel = np.empty((E, C, WSC), dtype=np.float16)
    w3pan = np.empty((E, C, 256), dtype=np.float16)
    for e in range(E):
        wpanel[e, :, 0:256] = W1[e].T
        w2t = W2[e].T
        wpanel[e, :, 256:512] = w2t[0:128, :]
        wpanel[e, :, 512:768] = w2t[128:256, :]
        w3inv = (W3[e] * inv[e][None, :]).T        # [HD, C]
        w3pan[e, :, 0:128] = w3inv[0:128, :]
        w3pan[e, :, 128:256] = w3inv[128:256, :]

    xr = x.reshape(B, C, P)
    in_maps = []
    orders = []      # core -> region -> sample
    slot_maps = []   # core -> {(sample, expert): slot}
    for c in range(NCORES):
        order = [int(ranks[SPC * q + c]) for q in range(SPC)]
        orders.append(order)
        slots = []
        slot_of = {}
        for q in range(SPC):
            s = order[q]
            es = esets[s] + [esets[s][0]] * (Rl[q] - len(esets[s]))
            for r, e in enumerate(es):
                if r < len(esets[s]):
                    slot_of[(s, e)] = len(slots)
                slots.append(e)
        slot_maps.append(slot_of)

        ws = np.empty((C, NSLOT * WSC), dtype=np.float16)
        w3 = np.empty((C, NSLOT * 256), dtype=np.float16)
        meta = np.zeros((C, MCOLS), dtype=np.float32)
        for d, e in enumerate(slots):
            ws[:, d * WSC:(d + 1) * WSC] = wpanel[e]
            w3[:, d * 256:(d + 1) * 256] = w3pan[e]
            meta[:, 4 * d + 0] = b1[e, 0:128]
            meta[:, 4 * d + 1] = b1[e, 128:256]
            meta[:, 4 * d + 2] = bAp[e, 0:128]
            meta[:, 4 * d + 3] = bAp[e, 128:256]

        im = {"meta": meta}
        base = 0
        for i, n in enumerate(XQ_CHUNKS):
            xc = np.empty((C, n * P), dtype=np.float16)
            for li in range(n):
                xc[:, li * P:(li + 1) * P] = xr[order[base + li]]
            im[f"xq{i}"] = xc
            base += n
        base = 0
        for i, n in enumerate(WCH):
            im[f"ws{i}"] = np.ascontiguousarray(
                ws[:, base * WSC:(base + n) * WSC])
            im[f"w3{i}"] = np.ascontiguousarray(
                w3[:, base * 256:(base + n) * 256])
            base += n
        in_maps.append(im)
    sched = (orders, slot_maps, Rkey, top_i, tw, b3)
    return (in_maps, sched), None


def combine_outputs(results, sched):
    """Host combine: y[g,s] = tw0*z[s,e0] + tw1*z[s,e1] + tw.b3."""
    orders, slot_maps, _Rkey, top_i, tw, b3 = sched
    core_of = {}
    for c in range(NCORES):
        for s in orders[c]:
            core_of[s] = c
    zs = [np.asarray(r["out"], dtype=np.float32) for r in results]
    outs = []
    for g in range(NG):
        og = np.empty((B, C, P), dtype=np.float32)
        for s in range(B):
            c = core_of[s]
            y = None
            for t in range(TOP):
                e = int(top_i[g, s, t])
                w = float(tw[g, s, t])
                zt = zs[c][slot_maps[c][(s, e)]]
                y = w * zt if y is None else y + w * zt
                if np.any(b3[e]):
                    y = y + w * b3[e][:, None]
            og[s] = y
        outs.append(og.reshape(B, C, H, W_))
    return tuple(outs)


def kernel(x, gates, W1, b1, W2, b2, bn_gamma, bn_beta, bn_mean, bn_var,
           W3, b3):
    from concourse.bass_utils import run_bass_kernel_spmd

    built, fb = build_in_maps({
        "x": x, "gates": gates, "W1": W1, "b1": b1, "W2": W2, "b2": b2,
        "bn_gamma": bn_gamma, "bn_beta": bn_beta, "bn_mean": bn_mean,
        "bn_var": bn_var, "W3": W3, "b3": b3,
    })
    if fb is not None:
        return fb
    in_maps, sched = built
    nc = _build_program(sched[2])
    res = run_bass_kernel_spmd(nc, in_maps, list(range(NCORES)))
    return combine_outputs(res.results, sched)
